# revision 45
# baseline (speedup 1.0000x reference)
"""Distance-correlation (DcorLoss) kernel for 8 trn2 NeuronCores.

Math: for x, y [n=8192, d=128]:
  a = pairwise_dist(x), b = pairwise_dist(y)   (n x n, symmetric, zero diag)
  A = double_center(a), B = double_center(b)
  dcor = -sqrt(sum(A*B)) / sqrt(sqrt(sum(A*A)) * sqrt(sum(B*B)))

Never materialize A/B:
  sum(A o B) = sum(a o b) - 2/n dot(rs_a, rs_b) + sum(a) sum(b) / n^2
and the squared-distance Frobenius norms have a closed form (host, exact):
  sum_ij dist^2 = 2n sum_i |x_i|^2 - 2 |sum_i x_i|^2
so the device only streams: row sums of a and b (ACT accum), column sums
(PE matmul with ones weights), and sum (a - mu) * b (DVE accum). All
combining is host fp64.

Default mode "symdr" stacks three structural tricks:

1. fp8 DoubleRow matmul (perf_mode=DoubleRow, K=256 virtual): plane 0
   carries the 128 data rows (-2 x_blk^T x gram), plane 1 rows 0..2 carry
   the column-norm hi/lo/lo2 splits against all-ones weight rows. One MM
   per 512-col psum half computes n_i-free sq distances entirely, halving
   PE streaming vs bf16 mains + K=2 norm matmuls.
2. Symmetry: core c computes only local windows 0..4 (its diagonal block
   + 4 cyclic neighbors) = 5/8 of the row-block work. Full-matrix sums
   use sum_full = 2*sum_computed - S(w=0) - S(w=4); full row sums add
   mirrored per-column sums of windows 1..3 (PE ones-matmuls into one
   psum bank at partitions 0/32/64/96), gathered on host.
3. Per-core COLUMN ROTATION: core c's column j is global (j + c*1024)
   mod n, so the diagonal lands in window 0 on every core and the SPMD
   program is identical; the mu^2 diagonal forcing (sqrt NaN-safety)
   costs 2 bf16 matmuls on window-0 tiles only.

Precision: the final sums cancel ~1e8 -> ~1e6, which amplifies any BIAS
~1e4x. bf16-rounded sqrt outputs carry E[delta] ~ -2e-4 -> 5% error, so:
products and row-sum accums run on fp32 ACT outputs (unbiased); only the
column-sum matmuls read separate bf16 copies; global Ra/Rb come from the
unbiased fp32 accum totals; the dot products are mean-centered, which
cancels the constant per-row bias of the mirrored column sums.

ACT (ScalarE) is the bottleneck: 80 sqrt passes at 1 elem/cycle/lane
is ~95 us; PE (DR mains + colsums) and DVE (products + bf16 casts)
overlap underneath.

All operand prep is host-side: inputs arrive as fp8/f32 in final layout;
no on-device casts / norm computation / big reductions.
"""

import numpy as np
import ml_dtypes

import concourse.bass as bass
import concourse.tile as tile
from concourse import bacc, mybir
from concourse.bass_utils import run_bass_kernel_spmd

P = 128            # partitions / d
N = 8192           # points
NCORES = 8
BLK = N // NCORES  # 1024 rows per core
CI_N = BLK // P    # 8 row chunks per core
W = 1024           # column window
JT_N = N // W      # 8 column windows
MU = 16.0          # ~E[pairwise dist] for randn d=128; any constant is exact
RES_W = 24

BF16 = ml_dtypes.bfloat16
F8 = ml_dtypes.float8_e4m3

DEFAULT_MODE = "symdr"
MODES = ("base", "dr", "symdr")
NW = 5             # symdr: local windows 0..4 (diag + 4 cyclic)
CS_ROWS = (0, 32, 64, 96)  # psum partitions for a_h0, a_h1, b_h0, b_h1

_programs = {}


NW_SYM = 5         # sym mode: windows 0..4 (diag + 4 cyclic) per core
NCOL = N  # dram moving-tensor width (full; sym mode reads first 5 windows)


def _build(mode: str):
    dt = mybir.dt
    f32 = dt.float32
    bf = dt.bfloat16
    A = mybir.AluOpType
    AF = mybir.ActivationFunctionType

    f8 = dt.float8e4

    nc = bacc.Bacc("TRN2", target_bir_lowering=False, debug=False,
                   num_devices=NCORES)

    if mode in ("dr", "symdr"):
        # fp8 DoubleRow: K=256 virtual; plane 0 = data rows, plane 1 rows
        # 0..2 = column-norm hi/lo/lo2 (weights = ones there), rest zero.
        # dram layout is window-major so per-window DMAs are contiguous
        # 2 KiB/partition lines instead of strided 1 KiB ones.
        dxm = nc.dram_tensor("xm", [P, JT_N, 2, W], f8,
                             kind="ExternalInput").ap()
        dym = nc.dram_tensor("ym", [P, JT_N, 2, W], f8,
                             kind="ExternalInput").ap()
        dwx = nc.dram_tensor("wx", [P, 2, BLK], f8, kind="ExternalInput").ap()
        dwy = nc.dram_tensor("wy", [P, 2, BLK], f8, kind="ExternalInput").ap()
    else:
        dxT = nc.dram_tensor("xT", [P, N], bf, kind="ExternalInput").ap()
        dyT = nc.dram_tensor("yT", [P, N], bf, kind="ExternalInput").ap()
        dxb = nc.dram_tensor("xblk2", [P, BLK], bf, kind="ExternalInput").ap()
        dyb = nc.dram_tensor("yblk2", [P, BLK], bf, kind="ExternalInput").ap()
        dnfx = nc.dram_tensor("nfx", [2, N], bf, kind="ExternalInput").ap()
        dnfy = nc.dram_tensor("nfy", [2, N], bf, kind="ExternalInput").ap()
    dnbx = nc.dram_tensor("nbx", [P, CI_N], f32, kind="ExternalInput").ap()
    dnby = nc.dram_tensor("nby", [P, CI_N], f32, kind="ExternalInput").ap()
    deye = nc.dram_tensor("eye128", [P, P], bf, kind="ExternalInput").ap()
    dew = nc.dram_tensor("eyewide", [P, 4 * 512], bf, kind="ExternalInput").ap()
    dout = nc.dram_tensor("out", [P, RES_W], f32, kind="ExternalOutput").ap()
    if mode == "symdr":
        dcols = nc.dram_tensor("cols", [P, 3 * 512], f32,
                               kind="ExternalOutput").ap()
        dst01 = [nc.dram_tensor(f"st{q}o", [P, CI_N * NW], f32,
                                kind="ExternalOutput").ap() for q in range(2)]
        dst2 = nc.dram_tensor("st2", [P, CI_N * NW], f32,
                              kind="ExternalOutput").ap()
    n_w = NW if mode == "symdr" else JT_N

    with tile.TileContext(nc) as tc:
        with tc.tile_pool(name="const", bufs=1) as cp, \
             tc.tile_pool(name="psum", bufs=3, space="PSUM") as pp, \
             tc.tile_pool(name="cspsum", bufs=2, space="PSUM") as csp, \
             tc.tile_pool(name="ab", bufs=3) as abp, \
             tc.tile_pool(name="trd", bufs=2) as trd:

            # ── persistent operands, DMA'd in final dtype/layout ──────
            if mode in ("dr", "symdr"):
                xm = cp.tile([P, 2, N], f8, tag="xm")
                ym = cp.tile([P, 2, N], f8, tag="ym")
                wx = cp.tile([P, 2, BLK], f8, tag="wx")
                wy = cp.tile([P, 2, BLK], f8, tag="wy")
            else:
                xTc = cp.tile([P, N], bf, tag="xTc")
                yTc = cp.tile([P, N], bf, tag="yTc")
                xblk2 = cp.tile([P, BLK], bf, tag="xblk2")
                yblk2 = cp.tile([P, BLK], bf, tag="yblk2")
                nfx = cp.tile([2, N], bf, tag="nfx")
                nfy = cp.tile([2, N], bf, tag="nfy")
            nbx = cp.tile([P, CI_N], f32, tag="nbx")
            nby = cp.tile([P, CI_N], f32, tag="nby")
            eye128 = cp.tile([P, P], bf, tag="eye128")
            eyew = cp.tile([P, 4 * 512], bf, tag="eyew")
            ones2 = cp.tile([2, P], bf, tag="ones2")
            nc.vector.memset(ones2[:], 1.0)

            res = cp.tile([P, RES_W], f32, tag="res")
            nc.vector.memset(res[:], 0.0)

            st = [cp.tile([P, CI_N * n_w], f32, tag=f"st{q}", name=f"st{q}")
                  for q in range(3)]
            if mode == "symdr":
                onesP = cp.tile([P, 1], bf, tag="onesP")
                nc.vector.memset(onesP[:], 1.0)
                colsave = cp.tile([P, 3 * 512], f32, tag="colsave")

            # PE warm-up on constant data: release the HAM clock-gate
            # before real matmuls start (cold runs stream at 1.2 GHz).
            # symdr skips it: ACT (not PE) is the bottleneck, so warm-up
            # only delays the first main matmuls behind the DMA wave.
            wur = cp.tile([2, 512], bf, tag="wur")
            nc.vector.memset(wur[:], 0.0)
            n_warm = 0 if mode == "symdr" else 24
            for _ in range(n_warm):
                wt = pp.tile([P, W], f32, tag="ps")
                nc.tensor.matmul(wt[:, 0:512], ones2[:], wur[:],
                                 start=True, stop=True)
            # trigger the sqrt ACT_TABLE_LOADs (~2.7us) during the DMA wait
            # instead of on the first real tile; same bias/accum signature
            # as the real tiles so every needed table set loads now
            tldu = cp.tile([1, 8], f32, tag="tldu")
            tlda = cp.tile([1, 1], f32, tag="tlda")
            tldb = cp.tile([1, 1], f32, tag="tldb")
            nc.vector.memset(tldb[:], 0.0)
            nc.scalar.activation(tldu[:], wur[0:1, 0:8], AF.Sqrt,
                                 bias=tldb[:], accum_out=tlda[:])

            # small operands first, then per-window slices of the big
            # moving tensors so window-0 compute starts ASAP
            if mode in ("dr", "symdr"):
                # first-window operands gate tile 0: ship them first, split
                # across the two HWDGE queues (sync + scalar)
                w_first = 1 if mode == "symdr" else 0
                sl0 = bass.ts(w_first, W)
                nc.sync.dma_start(wx[:], dwx[:])
                nc.scalar.dma_start(wy[:], dwy[:])
                nc.sync.dma_start(xm[:, :, sl0], dxm[:, w_first])
                nc.scalar.dma_start(ym[:, :, sl0], dym[:, w_first])
            else:
                nc.sync.dma_start(xblk2[:], dxb[:])
                nc.sync.dma_start(yblk2[:], dyb[:])
            nc.sync.dma_start(nbx[:], dnbx[:])
            nc.scalar.dma_start(nby[:], dnby[:])
            nc.sync.dma_start(eye128[:], deye[:])
            nc.scalar.dma_start(eyew[:], dew[:])
            if mode in ("dr", "symdr"):
                order = [2, 3, 4, 0] if mode == "symdr" else range(1, n_w)
                for w in order:
                    sl = bass.ts(w, W)
                    nc.sync.dma_start(xm[:, :, sl], dxm[:, w])
                    nc.scalar.dma_start(ym[:, :, sl], dym[:, w])
            else:
                nc.sync.dma_start(nfx[:], dnfx[:])
                nc.sync.dma_start(nfy[:], dnfy[:])
                for w in range(JT_N):
                    sl = bass.ts(w, W)
                    nc.sync.dma_start(xTc[:, sl], dxT[:, sl])
                    nc.sync.dma_start(yTc[:, sl], dyT[:, sl])

            # ── main loop ─────────────────────────────────────────────
            # symdr: window 1 first (no diag fix -> shortest dependency
            # chain to the first ACTIVATE); diag window 0 last, where the
            # PE has slack under the ACT-bound steady state
            w_order = [1, 2, 3, 4, 0] if mode == "symdr" else range(n_w)
            for w in w_order:
                do_cs = mode == "symdr" and 1 <= w <= 3
                if do_cs:
                    cs = csp.tile([P, 512], f32, tag="cs")
                for ci in range(CI_N):
                    col = ci * n_w + w
                    h_diag = ci // 4
                    psA = pp.tile([P, W], f32, tag="ps")
                    psB = pp.tile([P, W], f32, tag="ps")
                    if mode in ("dr", "symdr"):
                        # one fp8 DoubleRow matmul per half: data + norm
                        # rows in a single K=256 pass; diag fix right after
                        # its own psum's matmuls so the ACT dependency
                        # chain stays per-psum
                        for ps_, ww, mov in ((psA, wx, xm), (psB, wy, ym)):
                            for h in range(2):
                                last = not (w == 0 and h == h_diag)
                                nc.tensor.matmul(
                                    ps_[:, bass.ds(h * 512, 512)],
                                    ww[:, :, bass.ts(ci, P)],
                                    mov[:, :, bass.ds(w * W + h * 512, 512)],
                                    start=True, stop=last,
                                    perf_mode=mybir.MatmulPerfMode.DoubleRow)
                            if w == 0 and mode == "symdr":
                                nc.tensor.matmul(
                                    ps_[:, bass.ds(h_diag * 512, 512)],
                                    eye128[:],
                                    eyew[:, bass.ts(ci % 4, 512)],
                                    start=False, stop=True)
                    else:
                        # mains (grouped by weights), then norm rows (shared
                        # ones2 weights), then diag fix (window 0 only)
                        for ps_, blk2, full in ((psA, xblk2, xTc),
                                                (psB, yblk2, yTc)):
                            for h in range(2):
                                nc.tensor.matmul(
                                    ps_[:, bass.ds(h * 512, 512)],
                                    blk2[:, bass.ts(ci, P)],
                                    full[:, bass.ds(w * W + h * 512, 512)],
                                    start=True, stop=False)
                        for ps_, nf in ((psA, nfx), (psB, nfy)):
                            for h in range(2):
                                last = not (w == 0 and h == h_diag)
                                nc.tensor.matmul(
                                    ps_[:, bass.ds(h * 512, 512)], ones2[:],
                                    nf[:, bass.ds(w * W + h * 512, 512)],
                                    start=False, stop=last)
                    if w == 0 and mode != "symdr":
                        for ps_ in (psA, psB):
                            nc.tensor.matmul(
                                ps_[:, bass.ds(h_diag * 512, 512)],
                                eye128[:],
                                eyew[:, bass.ts(ci % 4, 512)],
                                start=False, stop=True)

                    aT = abp.tile([P, W], f32, tag="a")
                    bT = abp.tile([P, W], f32, tag="b")
                    nc.scalar.activation(aT[:], psA[:], AF.Sqrt,
                                         bias=nbx[:, ci:ci + 1],
                                         accum_out=st[0][:, col:col + 1])
                    nc.scalar.activation(bT[:], psB[:], AF.Sqrt,
                                         bias=nby[:, ci:ci + 1],
                                         accum_out=st[1][:, col:col + 1])
                    if do_cs:
                        # bf16 copies feed the column-sum matmuls; the fp32
                        # originals feed the product pass (bf16 rounding
                        # bias is amplified ~1e4x by cancellation in the
                        # final sums, so rs/pab paths must stay fp32)
                        a16 = trd.tile([P, W], bf, tag="a16")
                        b16 = trd.tile([P, W], bf, tag="b16")
                        nc.vector.tensor_copy(a16[:], aT[:])
                        nc.vector.tensor_copy(b16[:], bT[:])
                        # column sums for mirrored row sums: one psum bank,
                        # streams at psum partitions 0/32/64/96
                        for r, t_ in ((0, a16), (64, b16)):
                            for h in range(2):
                                nc.tensor.matmul(
                                    cs[r + 32 * h:r + 32 * h + 1, 0:512],
                                    onesP[:],
                                    t_[:, bass.ds(h * 512, 512)],
                                    start=(ci == 0), stop=(ci == CI_N - 1),
                                    tile_position=(0, r + 32 * h))
                    t0 = trd.tile([P, W], bf, tag="t")
                    nc.vector.scalar_tensor_tensor(
                        t0[:], aT[:], MU, bT[:], op0=A.subtract, op1=A.mult,
                        accum_out=st[2][:, col:col + 1])
                if do_cs:
                    csl = bass.ts(w - 1, 512)
                    nc.vector.tensor_copy(colsave[:, csl], cs[:])
                    nc.scalar.dma_start(dcols[:, csl], colsave[:, csl])

            # ── epilogue: fold windows, ship result ───────────────────
            if mode == "symdr":
                nc.sync.dma_start(dst01[0][:], st[0][:])
                nc.scalar.dma_start(dst01[1][:], st[1][:])
                nc.sync.dma_start(dst2[:], st[2][:])
                nc.scalar.dma_start(dout[:], res[:])
            else:
                for q in range(3):
                    for ci in range(CI_N):
                        o = q * CI_N + ci
                        nc.vector.tensor_reduce(res[:, o:o + 1],
                                                st[q][:, bass.ts(ci, n_w)],
                                                axis=mybir.AxisListType.X,
                                                op=A.add)
                nc.sync.dma_start(dout[:], res[:])

    nc.compile()
    return nc


def _get_program(mode: str):
    if mode not in _programs:
        _programs[mode] = _build(mode)
    return _programs[mode]


def make_in_maps(x: np.ndarray, y: np.ndarray, mode: str = "bf16"):
    x = np.asarray(x, np.float32)
    y = np.asarray(y, np.float32)
    QD = F8 if mode in ("dr", "symdr") else BF16
    xb = x.astype(QD)
    yb = y.astype(QD)

    eye = (np.eye(P) * MU).astype(BF16)
    ew = np.zeros((P, 4 * 512), BF16)
    for k in range(4):
        for p in range(P):
            ew[p, k * 512 + k * P + p] = BF16(MU)

    def norms_split(vb, parts):
        n64 = (vb.astype(np.float64) ** 2).sum(axis=1)
        rows, rem = [], n64.copy()
        for _ in range(parts):
            r = rem.astype(QD)
            rows.append(r)
            rem = rem - r.astype(np.float64)
        return n64, np.stack(rows).astype(QD)

    nparts = 3 if mode in ("dr", "symdr") else 2
    nx64, nfx = norms_split(xb, nparts)
    ny64, nfy = norms_split(yb, nparts)

    xT = np.ascontiguousarray(xb.T)           # [128, 8192]
    yT = np.ascontiguousarray(yb.T)
    xT2 = np.concatenate([xT, xT], axis=1)    # for cheap rotation slicing
    yT2 = np.concatenate([yT, yT], axis=1)
    nfx2 = np.concatenate([nfx, nfx], axis=1)
    nfy2 = np.concatenate([nfy, nfy], axis=1)

    in_maps = []
    for c in range(NCORES):
        o = c * BLK
        xTr = np.ascontiguousarray(xT2[:, o:o + N])
        yTr = np.ascontiguousarray(yT2[:, o:o + N])
        im = {
            "nbx": np.ascontiguousarray(
                nx64[o:o + BLK].reshape(CI_N, P).T.astype(np.float32)),
            "nby": np.ascontiguousarray(
                ny64[o:o + BLK].reshape(CI_N, P).T.astype(np.float32)),
            "eye128": eye,
            "eyewide": ew,
        }
        if mode in ("dr", "symdr"):
            def moving(vT, nf2):
                m = np.zeros((P, 2, N), QD)
                m[:, 0, :] = vT
                m[0:3, 1, :] = nf2[:, o:o + N]
                # window-major dram layout: [P, JT_N, 2, W]
                return np.ascontiguousarray(
                    m.reshape(P, 2, JT_N, W).transpose(0, 2, 1, 3))

            def weights(vT):
                ww = np.zeros((P, 2, BLK), QD)
                ww[:, 0, :] = QD(-2.0) * vT[:, 0:BLK]
                ww[0:3, 1, :] = QD(1.0)
                return ww

            im.update({"xm": moving(xTr, nfx2), "ym": moving(yTr, nfy2),
                       "wx": weights(xTr), "wy": weights(yTr)})
        else:
            im.update({
                "xT": xTr, "yT": yTr,
                "xblk2": np.ascontiguousarray(BF16(-2.0) * xTr[:, 0:BLK]),
                "yblk2": np.ascontiguousarray(BF16(-2.0) * yTr[:, 0:BLK]),
                "nfx": np.ascontiguousarray(nfx2[:, o:o + N]),
                "nfy": np.ascontiguousarray(nfy2[:, o:o + N]),
            })
        in_maps.append(im)
    host = {"xb64": xb.astype(np.float64), "yb64": yb.astype(np.float64)}
    return in_maps, host


def finalize(outs, host, mode="base", colss=None, st2s=None):
    """outs: 8 x [128, 24] f32 -> scalar dcor (host fp64).

    Cols per core: rs_a 0:8 | rs_b 8:16 | pab 16:24, laid out [p, ci] for
    global row c*1024 + ci*128 + p. Device row sums include the forced
    diagonal entry sqrt(mu^2) = mu (true diag of a distance matrix is 0).

    symdr: each core computed local windows 0..4 only. Full sums over the
    symmetric matrices: sum_full = 2*sum_computed - S(w=0) - S(w=4); full
    row sums add mirrored column sums from windows 1..3 of cores bi-1..bi-3.
    """
    n = float(N)
    rs_a = np.empty(N, np.float64)
    rs_b = np.empty(N, np.float64)
    pab = 0.0

    # exact Frobenius norms of the quantized-point distance matrices
    def sq_frob(v64):
        s = v64.sum(axis=0)
        return 2.0 * n * (v64 * v64).sum() - 2.0 * np.dot(s, s)

    sq_a = sq_frob(host["xb64"])
    sq_b = sq_frob(host["yb64"])

    if mode == "symdr":
        # per-row strip sums + per-window totals (for 2*S - S0 - S4)
        tot = np.zeros((2, 3), np.float64)  # [a/b][all, w0, w4]
        for c in range(NCORES):
            for q, rs in ((0, rs_a), (1, rs_b)):
                sq_ = np.asarray(st2s[c][q], np.float64).reshape(P, CI_N, NW)
                rs[c * BLK:(c + 1) * BLK] = (
                    sq_.sum(axis=2).T.ravel())
                tot[q] += (sq_.sum(), sq_[:, :, 0].sum(), sq_[:, :, 4].sum())
        # mirrored row-sum contributions from columns of windows 1..3
        VA = np.empty((NCORES, 3, BLK), np.float64)
        VB = np.empty((NCORES, 3, BLK), np.float64)
        for c, cl in enumerate(colss):
            cl = np.asarray(cl, np.float64)
            for wp in range(1, 4):
                sl = slice((wp - 1) * 512, wp * 512)
                VA[c, wp - 1] = np.concatenate([cl[0, sl], cl[32, sl]])
                VB[c, wp - 1] = np.concatenate([cl[64, sl], cl[96, sl]])
        for bi in range(NCORES):
            for wp in range(1, 4):
                rs_a[bi * BLK:(bi + 1) * BLK] += VA[(bi - wp) % NCORES, wp - 1]
                rs_b[bi * BLK:(bi + 1) * BLK] += VB[(bi - wp) % NCORES, wp - 1]
        # pab over the full matrix: 2*computed - S(w0) - S(w4)
        p_all = p_w0 = p_w4 = 0.0
        for s2 in [st2s[c][2] for c in range(NCORES)]:
            s2 = np.asarray(s2, np.float64).reshape(P, CI_N, NW)
            p_all += s2.sum()
            p_w0 += s2[:, :, 0].sum()
            p_w4 += s2[:, :, 4].sum()
        pab = 2.0 * p_all - p_w0 - p_w4
        # unbiased global sums (fp32 accum path; removes forced diag)
        Ra = 2.0 * tot[0, 0] - tot[0, 1] - tot[0, 2] - n * MU
        Rb = 2.0 * tot[1, 0] - tot[1, 1] - tot[1, 2] - n * MU
        sa = rs_a - MU         # per-row (mirror part carries tiny bf16 bias;
        sb = rs_b - MU         # centered dots below are immune to it)
        sat = sa - Ra / n
        sbt = sb - Rb / n
        sum_ab = pab + MU * Rb
        sumAB = sum_ab - 2.0 * np.dot(sat, sbt) / n - Ra * Rb / n**2
        sumAA = sq_a - 2.0 * np.dot(sat, sat) / n - Ra * Ra / n**2
        sumBB = sq_b - 2.0 * np.dot(sbt, sbt) / n - Rb * Rb / n**2
        inv_n2 = 1.0 / (n * n)
        return np.asarray(
            -np.sqrt(sumAB * inv_n2)
            / np.sqrt(np.sqrt(sumAA * inv_n2) * np.sqrt(sumBB * inv_n2)),
            dtype=np.float32)

    for c, o in enumerate(outs):
        o = np.asarray(o, np.float64)
        rs_a[c * BLK:(c + 1) * BLK] = o[:, 0:CI_N].T.ravel()
        rs_b[c * BLK:(c + 1) * BLK] = o[:, CI_N:2 * CI_N].T.ravel()
        pab += o[:, 2 * CI_N:3 * CI_N].sum()

    sa = rs_a - MU             # true (zero-diag) row sums of a
    sb = rs_b - MU
    Ra = sa.sum()
    Rb = sb.sum()

    # device pab = sum (a - mu) * b; forced diag contributes (mu-mu)*mu = 0,
    # matching the true diag contribution (0 - mu) * 0 = 0 exactly.
    sum_ab = pab + MU * Rb

    sumAB = sum_ab - 2.0 * np.dot(sa, sb) / n + Ra * Rb / n**2
    sumAA = sq_a - 2.0 * np.dot(sa, sa) / n + Ra * Ra / n**2
    sumBB = sq_b - 2.0 * np.dot(sb, sb) / n + Rb * Rb / n**2

    inv_n2 = 1.0 / (n * n)
    dcor = (-np.sqrt(sumAB * inv_n2)
            / np.sqrt(np.sqrt(sumAA * inv_n2) * np.sqrt(sumBB * inv_n2)))
    return np.asarray(dcor, dtype=np.float32)


def run(x, y, mm_mode=None, trace=False, tmpdir=None):
    mode = mm_mode if mm_mode in MODES else DEFAULT_MODE
    nc = _get_program(mode)
    in_maps, host = make_in_maps(x, y, mode)
    res = run_bass_kernel_spmd(nc, in_maps, core_ids=list(range(NCORES)),
                               trace=trace, tmpdir=tmpdir)
    outs = [r["out"] for r in res.results]
    colss = [r["cols"] for r in res.results] if mode == "symdr" else None
    st2s = ([(r["st0o"], r["st1o"], r["st2"]) for r in res.results]
            if mode == "symdr" else None)
    return finalize(outs, host, mode, colss, st2s), res


def kernel(x, y):
    val, _ = run(x, y)
    return val


# revision 47
# speedup vs baseline: 1.0111x; 1.0111x over previous
"""Distance-correlation (DcorLoss) kernel for 8 trn2 NeuronCores.

Math: for x, y [n=8192, d=128]:
  a = pairwise_dist(x), b = pairwise_dist(y)   (n x n, symmetric, zero diag)
  A = double_center(a), B = double_center(b)
  dcor = -sqrt(sum(A*B)) / sqrt(sqrt(sum(A*A)) * sqrt(sum(B*B)))

Never materialize A/B:
  sum(A o B) = sum(a o b) - 2/n dot(rs_a, rs_b) + sum(a) sum(b) / n^2
and the squared-distance Frobenius norms have a closed form (host, exact):
  sum_ij dist^2 = 2n sum_i |x_i|^2 - 2 |sum_i x_i|^2
so the device only streams: row sums of a and b (ACT accum), column sums
(PE matmul with ones weights), and sum (a - mu) * b (DVE accum). All
combining is host fp64.

Default mode "symdr" stacks three structural tricks:

1. fp8 DoubleRow matmul (perf_mode=DoubleRow, K=256 virtual): plane 0
   carries the 128 data rows (-2 x_blk^T x gram), plane 1 rows 0..2 carry
   the column-norm hi/lo/lo2 splits against all-ones weight rows. One MM
   per 512-col psum half computes n_i-free sq distances entirely, halving
   PE streaming vs bf16 mains + K=2 norm matmuls.
2. Symmetry: core c computes only local windows 0..4 (its diagonal block
   + 4 cyclic neighbors) = 5/8 of the row-block work. Full-matrix sums
   use sum_full = 2*sum_computed - S(w=0) - S(w=4); full row sums add
   mirrored per-column sums of windows 1..3 (PE ones-matmuls into one
   psum bank at partitions 0/32/64/96), gathered on host.
3. Per-core COLUMN ROTATION: core c's column j is global (j + c*1024)
   mod n, so the diagonal lands in window 0 on every core and the SPMD
   program is identical; the mu^2 diagonal forcing (sqrt NaN-safety)
   costs 2 bf16 matmuls on window-0 tiles only.

Precision: the final sums cancel ~1e8 -> ~1e6, which amplifies any BIAS
~1e4x. bf16-rounded sqrt outputs carry E[delta] ~ -2e-4 -> 5% error, so:
products and row-sum accums run on fp32 ACT outputs (unbiased); only the
column-sum matmuls read separate bf16 copies; global Ra/Rb come from the
unbiased fp32 accum totals; the dot products are mean-centered, which
cancels the constant per-row bias of the mirrored column sums.

ACT (ScalarE) is the bottleneck: 80 sqrt passes at 1 elem/cycle/lane
is ~95 us; PE (DR mains + colsums) and DVE (products + bf16 casts)
overlap underneath.

All operand prep is host-side: inputs arrive as fp8/f32 in final layout;
no on-device casts / norm computation / big reductions.
"""

import numpy as np
import ml_dtypes

import concourse.bass as bass
import concourse.tile as tile
from concourse import bacc, mybir
from concourse.bass_utils import run_bass_kernel_spmd

P = 128            # partitions / d
N = 8192           # points
NCORES = 8
BLK = N // NCORES  # 1024 rows per core
CI_N = BLK // P    # 8 row chunks per core
W = 1024           # column window
JT_N = N // W      # 8 column windows
MU = 16.0          # ~E[pairwise dist] for randn d=128; any constant is exact
RES_W = 24

BF16 = ml_dtypes.bfloat16
F8 = ml_dtypes.float8_e4m3

DEFAULT_MODE = "symdr"
MODES = ("base", "dr", "symdr")
NW = 5             # symdr: local windows 0..4 (diag + 4 cyclic)
CS_ROWS = (0, 32, 64, 96)  # psum partitions for a_h0, a_h1, b_h0, b_h1

_programs = {}


NW_SYM = 5         # sym mode: windows 0..4 (diag + 4 cyclic) per core
NCOL = N  # dram moving-tensor width (full; sym mode reads first 5 windows)


def _build(mode: str):
    dt = mybir.dt
    f32 = dt.float32
    bf = dt.bfloat16
    A = mybir.AluOpType
    AF = mybir.ActivationFunctionType

    f8 = dt.float8e4

    nc = bacc.Bacc("TRN2", target_bir_lowering=False, debug=False,
                   num_devices=NCORES)

    if mode in ("dr", "symdr"):
        # fp8 DoubleRow: K=256 virtual; plane 0 = data rows, plane 1 rows
        # 0..2 = column-norm hi/lo/lo2 (weights = ones there), rest zero.
        # dram layout is window-major so per-window DMAs are contiguous
        # 2 KiB/partition lines instead of strided 1 KiB ones.
        dxm = nc.dram_tensor("xm", [P, JT_N, 2, W], f8,
                             kind="ExternalInput").ap()
        dym = nc.dram_tensor("ym", [P, JT_N, 2, W], f8,
                             kind="ExternalInput").ap()
        dwx = nc.dram_tensor("wx", [P, 2, BLK], f8, kind="ExternalInput").ap()
        dwy = nc.dram_tensor("wy", [P, 2, BLK], f8, kind="ExternalInput").ap()
    else:
        dxT = nc.dram_tensor("xT", [P, N], bf, kind="ExternalInput").ap()
        dyT = nc.dram_tensor("yT", [P, N], bf, kind="ExternalInput").ap()
        dxb = nc.dram_tensor("xblk2", [P, BLK], bf, kind="ExternalInput").ap()
        dyb = nc.dram_tensor("yblk2", [P, BLK], bf, kind="ExternalInput").ap()
        dnfx = nc.dram_tensor("nfx", [2, N], bf, kind="ExternalInput").ap()
        dnfy = nc.dram_tensor("nfy", [2, N], bf, kind="ExternalInput").ap()
    dnbx = nc.dram_tensor("nbx", [P, CI_N], f32, kind="ExternalInput").ap()
    dnby = nc.dram_tensor("nby", [P, CI_N], f32, kind="ExternalInput").ap()
    deye = nc.dram_tensor("eye128", [P, P], bf, kind="ExternalInput").ap()
    dew = nc.dram_tensor("eyewide", [P, 4 * 512], bf, kind="ExternalInput").ap()
    dout = nc.dram_tensor("out", [P, RES_W], f32, kind="ExternalOutput").ap()
    if mode == "symdr":
        dcols = nc.dram_tensor("cols", [P, 3 * 512], f32,
                               kind="ExternalOutput").ap()
        dst01 = [nc.dram_tensor(f"st{q}o", [P, CI_N * NW], f32,
                                kind="ExternalOutput").ap() for q in range(2)]
        dst2 = nc.dram_tensor("st2", [P, CI_N * NW], f32,
                              kind="ExternalOutput").ap()
    n_w = NW if mode == "symdr" else JT_N

    with tile.TileContext(nc) as tc:
        with tc.tile_pool(name="const", bufs=1) as cp, \
             tc.tile_pool(name="psum", bufs=3, space="PSUM") as pp, \
             tc.tile_pool(name="cspsum", bufs=2, space="PSUM") as csp, \
             tc.tile_pool(name="ab", bufs=3) as abp, \
             tc.tile_pool(name="trd", bufs=2) as trd:

            # ── persistent operands, DMA'd in final dtype/layout ──────
            if mode in ("dr", "symdr"):
                xm = cp.tile([P, 2, N], f8, tag="xm")
                ym = cp.tile([P, 2, N], f8, tag="ym")
                wx = cp.tile([P, 2, BLK], f8, tag="wx")
                wy = cp.tile([P, 2, BLK], f8, tag="wy")
            else:
                xTc = cp.tile([P, N], bf, tag="xTc")
                yTc = cp.tile([P, N], bf, tag="yTc")
                xblk2 = cp.tile([P, BLK], bf, tag="xblk2")
                yblk2 = cp.tile([P, BLK], bf, tag="yblk2")
                nfx = cp.tile([2, N], bf, tag="nfx")
                nfy = cp.tile([2, N], bf, tag="nfy")
            nbx = cp.tile([P, CI_N], f32, tag="nbx")
            nby = cp.tile([P, CI_N], f32, tag="nby")
            eye128 = cp.tile([P, P], bf, tag="eye128")
            eyew = cp.tile([P, 4 * 512], bf, tag="eyew")
            ones2 = cp.tile([2, P], bf, tag="ones2")
            nc.vector.memset(ones2[:], 1.0)

            res = cp.tile([P, RES_W], f32, tag="res")
            nc.vector.memset(res[:], 0.0)

            st = [cp.tile([P, CI_N * n_w], f32, tag=f"st{q}", name=f"st{q}")
                  for q in range(3)]
            if mode == "symdr":
                onesP = cp.tile([P, 1], bf, tag="onesP")
                nc.vector.memset(onesP[:], 1.0)
                colsave = cp.tile([P, 3 * 512], f32, tag="colsave")

            # PE warm-up on constant data: release the HAM clock-gate
            # before real matmuls start (cold runs stream at 1.2 GHz).
            # symdr skips it: ACT (not PE) is the bottleneck, so warm-up
            # only delays the first main matmuls behind the DMA wave.
            wur = cp.tile([2, 512], bf, tag="wur")
            nc.vector.memset(wur[:], 0.0)
            n_warm = 0 if mode == "symdr" else 24
            for _ in range(n_warm):
                wt = pp.tile([P, W], f32, tag="ps")
                nc.tensor.matmul(wt[:, 0:512], ones2[:], wur[:],
                                 start=True, stop=True)
            # trigger the sqrt ACT_TABLE_LOADs (~2.7us) during the DMA wait
            # instead of on the first real tile; same bias/accum signature
            # as the real tiles so every needed table set loads now
            tldu = cp.tile([1, 8], f32, tag="tldu")
            tlda = cp.tile([1, 1], f32, tag="tlda")
            tldb = cp.tile([1, 1], f32, tag="tldb")
            nc.vector.memset(tldb[:], 0.0)
            nc.scalar.activation(tldu[:], wur[0:1, 0:8], AF.Sqrt,
                                 bias=tldb[:], accum_out=tlda[:])

            # small operands first, then per-window slices of the big
            # moving tensors so window-0 compute starts ASAP
            if mode in ("dr", "symdr"):
                # the scalar queue carries the ACTIVATE stream: only the
                # few transfers that gate tile 0 may dispatch there; all
                # bulk DMAs go on sync so their ~640ns dispatches never
                # delay the first sqrt
                w_first = 1 if mode == "symdr" else 0
                sl0 = bass.ts(w_first, W)
                nc.sync.dma_start(wx[:], dwx[:])
                nc.scalar.dma_start(wy[:], dwy[:])
                nc.sync.dma_start(xm[:, :, sl0], dxm[:, w_first])
                nc.scalar.dma_start(ym[:, :, sl0], dym[:, w_first])
            else:
                nc.sync.dma_start(xblk2[:], dxb[:])
                nc.sync.dma_start(yblk2[:], dyb[:])
            nc.sync.dma_start(nbx[:], dnbx[:])
            nc.scalar.dma_start(nby[:], dnby[:])
            nc.sync.dma_start(eye128[:], deye[:])
            if mode in ("dr", "symdr"):
                order = [2, 3, 4, 0] if mode == "symdr" else range(1, n_w)
                for w in order:
                    sl = bass.ts(w, W)
                    nc.sync.dma_start(xm[:, :, sl], dxm[:, w])
                    nc.sync.dma_start(ym[:, :, sl], dym[:, w])
                nc.sync.dma_start(eyew[:], dew[:])
            else:
                nc.sync.dma_start(eyew[:], dew[:])
                nc.sync.dma_start(nfx[:], dnfx[:])
                nc.sync.dma_start(nfy[:], dnfy[:])
                for w in range(JT_N):
                    sl = bass.ts(w, W)
                    nc.sync.dma_start(xTc[:, sl], dxT[:, sl])
                    nc.sync.dma_start(yTc[:, sl], dyT[:, sl])

            # ── main loop ─────────────────────────────────────────────
            # symdr: window 1 first (no diag fix -> shortest dependency
            # chain to the first ACTIVATE); diag window 0 last, where the
            # PE has slack under the ACT-bound steady state
            w_order = [1, 2, 3, 4, 0] if mode == "symdr" else range(n_w)
            for w in w_order:
                do_cs = mode == "symdr" and 1 <= w <= 3
                if do_cs:
                    cs = csp.tile([P, 512], f32, tag="cs")
                for ci in range(CI_N):
                    col = ci * n_w + w
                    h_diag = ci // 4
                    psA = pp.tile([P, W], f32, tag="ps")
                    psB = pp.tile([P, W], f32, tag="ps")
                    if mode in ("dr", "symdr"):
                        # one fp8 DoubleRow matmul per half: data + norm
                        # rows in a single K=256 pass; diag fix right after
                        # its own psum's matmuls so the ACT dependency
                        # chain stays per-psum
                        for ps_, ww, mov in ((psA, wx, xm), (psB, wy, ym)):
                            for h in range(2):
                                last = not (w == 0 and h == h_diag)
                                nc.tensor.matmul(
                                    ps_[:, bass.ds(h * 512, 512)],
                                    ww[:, :, bass.ts(ci, P)],
                                    mov[:, :, bass.ds(w * W + h * 512, 512)],
                                    start=True, stop=last,
                                    perf_mode=mybir.MatmulPerfMode.DoubleRow)
                            if w == 0 and mode == "symdr":
                                nc.tensor.matmul(
                                    ps_[:, bass.ds(h_diag * 512, 512)],
                                    eye128[:],
                                    eyew[:, bass.ts(ci % 4, 512)],
                                    start=False, stop=True)
                    else:
                        # mains (grouped by weights), then norm rows (shared
                        # ones2 weights), then diag fix (window 0 only)
                        for ps_, blk2, full in ((psA, xblk2, xTc),
                                                (psB, yblk2, yTc)):
                            for h in range(2):
                                nc.tensor.matmul(
                                    ps_[:, bass.ds(h * 512, 512)],
                                    blk2[:, bass.ts(ci, P)],
                                    full[:, bass.ds(w * W + h * 512, 512)],
                                    start=True, stop=False)
                        for ps_, nf in ((psA, nfx), (psB, nfy)):
                            for h in range(2):
                                last = not (w == 0 and h == h_diag)
                                nc.tensor.matmul(
                                    ps_[:, bass.ds(h * 512, 512)], ones2[:],
                                    nf[:, bass.ds(w * W + h * 512, 512)],
                                    start=False, stop=last)
                    if w == 0 and mode != "symdr":
                        for ps_ in (psA, psB):
                            nc.tensor.matmul(
                                ps_[:, bass.ds(h_diag * 512, 512)],
                                eye128[:],
                                eyew[:, bass.ts(ci % 4, 512)],
                                start=False, stop=True)

                    aT = abp.tile([P, W], f32, tag="a")
                    bT = abp.tile([P, W], f32, tag="b")
                    nc.scalar.activation(aT[:], psA[:], AF.Sqrt,
                                         bias=nbx[:, ci:ci + 1],
                                         accum_out=st[0][:, col:col + 1])
                    nc.scalar.activation(bT[:], psB[:], AF.Sqrt,
                                         bias=nby[:, ci:ci + 1],
                                         accum_out=st[1][:, col:col + 1])
                    if do_cs:
                        # bf16 copies feed the column-sum matmuls; the fp32
                        # originals feed the product pass (bf16 rounding
                        # bias is amplified ~1e4x by cancellation in the
                        # final sums, so rs/pab paths must stay fp32)
                        a16 = trd.tile([P, W], bf, tag="a16")
                        b16 = trd.tile([P, W], bf, tag="b16")
                        nc.vector.tensor_copy(a16[:], aT[:])
                        nc.vector.tensor_copy(b16[:], bT[:])
                        # column sums for mirrored row sums: one psum bank,
                        # streams at psum partitions 0/32/64/96
                        for r, t_ in ((0, a16), (64, b16)):
                            for h in range(2):
                                nc.tensor.matmul(
                                    cs[r + 32 * h:r + 32 * h + 1, 0:512],
                                    onesP[:],
                                    t_[:, bass.ds(h * 512, 512)],
                                    start=(ci == 0), stop=(ci == CI_N - 1),
                                    tile_position=(0, r + 32 * h))
                    t0 = trd.tile([P, W], bf, tag="t")
                    nc.vector.scalar_tensor_tensor(
                        t0[:], aT[:], MU, bT[:], op0=A.subtract, op1=A.mult,
                        accum_out=st[2][:, col:col + 1])
                if do_cs:
                    csl = bass.ts(w - 1, 512)
                    nc.vector.tensor_copy(colsave[:, csl], cs[:])
                    nc.sync.dma_start(dcols[:, csl], colsave[:, csl])

            # ── epilogue: fold windows, ship result ───────────────────
            if mode == "symdr":
                nc.sync.dma_start(dst01[0][:], st[0][:])
                nc.scalar.dma_start(dst01[1][:], st[1][:])
                nc.sync.dma_start(dst2[:], st[2][:])
                nc.scalar.dma_start(dout[:], res[:])
            else:
                for q in range(3):
                    for ci in range(CI_N):
                        o = q * CI_N + ci
                        nc.vector.tensor_reduce(res[:, o:o + 1],
                                                st[q][:, bass.ts(ci, n_w)],
                                                axis=mybir.AxisListType.X,
                                                op=A.add)
                nc.sync.dma_start(dout[:], res[:])

    nc.compile()
    return nc


def _get_program(mode: str):
    if mode not in _programs:
        _programs[mode] = _build(mode)
    return _programs[mode]


def make_in_maps(x: np.ndarray, y: np.ndarray, mode: str = "bf16"):
    x = np.asarray(x, np.float32)
    y = np.asarray(y, np.float32)
    QD = F8 if mode in ("dr", "symdr") else BF16
    xb = x.astype(QD)
    yb = y.astype(QD)

    eye = (np.eye(P) * MU).astype(BF16)
    ew = np.zeros((P, 4 * 512), BF16)
    for k in range(4):
        for p in range(P):
            ew[p, k * 512 + k * P + p] = BF16(MU)

    def norms_split(vb, parts):
        n64 = (vb.astype(np.float64) ** 2).sum(axis=1)
        rows, rem = [], n64.copy()
        for _ in range(parts):
            r = rem.astype(QD)
            rows.append(r)
            rem = rem - r.astype(np.float64)
        return n64, np.stack(rows).astype(QD)

    nparts = 3 if mode in ("dr", "symdr") else 2
    nx64, nfx = norms_split(xb, nparts)
    ny64, nfy = norms_split(yb, nparts)

    xT = np.ascontiguousarray(xb.T)           # [128, 8192]
    yT = np.ascontiguousarray(yb.T)
    xT2 = np.concatenate([xT, xT], axis=1)    # for cheap rotation slicing
    yT2 = np.concatenate([yT, yT], axis=1)
    nfx2 = np.concatenate([nfx, nfx], axis=1)
    nfy2 = np.concatenate([nfy, nfy], axis=1)

    in_maps = []
    for c in range(NCORES):
        o = c * BLK
        xTr = np.ascontiguousarray(xT2[:, o:o + N])
        yTr = np.ascontiguousarray(yT2[:, o:o + N])
        im = {
            "nbx": np.ascontiguousarray(
                nx64[o:o + BLK].reshape(CI_N, P).T.astype(np.float32)),
            "nby": np.ascontiguousarray(
                ny64[o:o + BLK].reshape(CI_N, P).T.astype(np.float32)),
            "eye128": eye,
            "eyewide": ew,
        }
        if mode in ("dr", "symdr"):
            def moving(vT, nf2):
                m = np.zeros((P, 2, N), QD)
                m[:, 0, :] = vT
                m[0:3, 1, :] = nf2[:, o:o + N]
                # window-major dram layout: [P, JT_N, 2, W]
                return np.ascontiguousarray(
                    m.reshape(P, 2, JT_N, W).transpose(0, 2, 1, 3))

            def weights(vT):
                ww = np.zeros((P, 2, BLK), QD)
                ww[:, 0, :] = QD(-2.0) * vT[:, 0:BLK]
                ww[0:3, 1, :] = QD(1.0)
                return ww

            im.update({"xm": moving(xTr, nfx2), "ym": moving(yTr, nfy2),
                       "wx": weights(xTr), "wy": weights(yTr)})
        else:
            im.update({
                "xT": xTr, "yT": yTr,
                "xblk2": np.ascontiguousarray(BF16(-2.0) * xTr[:, 0:BLK]),
                "yblk2": np.ascontiguousarray(BF16(-2.0) * yTr[:, 0:BLK]),
                "nfx": np.ascontiguousarray(nfx2[:, o:o + N]),
                "nfy": np.ascontiguousarray(nfy2[:, o:o + N]),
            })
        in_maps.append(im)
    host = {"xb64": xb.astype(np.float64), "yb64": yb.astype(np.float64)}
    return in_maps, host


def finalize(outs, host, mode="base", colss=None, st2s=None):
    """outs: 8 x [128, 24] f32 -> scalar dcor (host fp64).

    Cols per core: rs_a 0:8 | rs_b 8:16 | pab 16:24, laid out [p, ci] for
    global row c*1024 + ci*128 + p. Device row sums include the forced
    diagonal entry sqrt(mu^2) = mu (true diag of a distance matrix is 0).

    symdr: each core computed local windows 0..4 only. Full sums over the
    symmetric matrices: sum_full = 2*sum_computed - S(w=0) - S(w=4); full
    row sums add mirrored column sums from windows 1..3 of cores bi-1..bi-3.
    """
    n = float(N)
    rs_a = np.empty(N, np.float64)
    rs_b = np.empty(N, np.float64)
    pab = 0.0

    # exact Frobenius norms of the quantized-point distance matrices
    def sq_frob(v64):
        s = v64.sum(axis=0)
        return 2.0 * n * (v64 * v64).sum() - 2.0 * np.dot(s, s)

    sq_a = sq_frob(host["xb64"])
    sq_b = sq_frob(host["yb64"])

    if mode == "symdr":
        # per-row strip sums + per-window totals (for 2*S - S0 - S4)
        tot = np.zeros((2, 3), np.float64)  # [a/b][all, w0, w4]
        for c in range(NCORES):
            for q, rs in ((0, rs_a), (1, rs_b)):
                sq_ = np.asarray(st2s[c][q], np.float64).reshape(P, CI_N, NW)
                rs[c * BLK:(c + 1) * BLK] = (
                    sq_.sum(axis=2).T.ravel())
                tot[q] += (sq_.sum(), sq_[:, :, 0].sum(), sq_[:, :, 4].sum())
        # mirrored row-sum contributions from columns of windows 1..3
        VA = np.empty((NCORES, 3, BLK), np.float64)
        VB = np.empty((NCORES, 3, BLK), np.float64)
        for c, cl in enumerate(colss):
            cl = np.asarray(cl, np.float64)
            for wp in range(1, 4):
                sl = slice((wp - 1) * 512, wp * 512)
                VA[c, wp - 1] = np.concatenate([cl[0, sl], cl[32, sl]])
                VB[c, wp - 1] = np.concatenate([cl[64, sl], cl[96, sl]])
        for bi in range(NCORES):
            for wp in range(1, 4):
                rs_a[bi * BLK:(bi + 1) * BLK] += VA[(bi - wp) % NCORES, wp - 1]
                rs_b[bi * BLK:(bi + 1) * BLK] += VB[(bi - wp) % NCORES, wp - 1]
        # pab over the full matrix: 2*computed - S(w0) - S(w4)
        p_all = p_w0 = p_w4 = 0.0
        for s2 in [st2s[c][2] for c in range(NCORES)]:
            s2 = np.asarray(s2, np.float64).reshape(P, CI_N, NW)
            p_all += s2.sum()
            p_w0 += s2[:, :, 0].sum()
            p_w4 += s2[:, :, 4].sum()
        pab = 2.0 * p_all - p_w0 - p_w4
        # unbiased global sums (fp32 accum path; removes forced diag)
        Ra = 2.0 * tot[0, 0] - tot[0, 1] - tot[0, 2] - n * MU
        Rb = 2.0 * tot[1, 0] - tot[1, 1] - tot[1, 2] - n * MU
        sa = rs_a - MU         # per-row (mirror part carries tiny bf16 bias;
        sb = rs_b - MU         # centered dots below are immune to it)
        sat = sa - Ra / n
        sbt = sb - Rb / n
        sum_ab = pab + MU * Rb
        sumAB = sum_ab - 2.0 * np.dot(sat, sbt) / n - Ra * Rb / n**2
        sumAA = sq_a - 2.0 * np.dot(sat, sat) / n - Ra * Ra / n**2
        sumBB = sq_b - 2.0 * np.dot(sbt, sbt) / n - Rb * Rb / n**2
        inv_n2 = 1.0 / (n * n)
        return np.asarray(
            -np.sqrt(sumAB * inv_n2)
            / np.sqrt(np.sqrt(sumAA * inv_n2) * np.sqrt(sumBB * inv_n2)),
            dtype=np.float32)

    for c, o in enumerate(outs):
        o = np.asarray(o, np.float64)
        rs_a[c * BLK:(c + 1) * BLK] = o[:, 0:CI_N].T.ravel()
        rs_b[c * BLK:(c + 1) * BLK] = o[:, CI_N:2 * CI_N].T.ravel()
        pab += o[:, 2 * CI_N:3 * CI_N].sum()

    sa = rs_a - MU             # true (zero-diag) row sums of a
    sb = rs_b - MU
    Ra = sa.sum()
    Rb = sb.sum()

    # device pab = sum (a - mu) * b; forced diag contributes (mu-mu)*mu = 0,
    # matching the true diag contribution (0 - mu) * 0 = 0 exactly.
    sum_ab = pab + MU * Rb

    sumAB = sum_ab - 2.0 * np.dot(sa, sb) / n + Ra * Rb / n**2
    sumAA = sq_a - 2.0 * np.dot(sa, sa) / n + Ra * Ra / n**2
    sumBB = sq_b - 2.0 * np.dot(sb, sb) / n + Rb * Rb / n**2

    inv_n2 = 1.0 / (n * n)
    dcor = (-np.sqrt(sumAB * inv_n2)
            / np.sqrt(np.sqrt(sumAA * inv_n2) * np.sqrt(sumBB * inv_n2)))
    return np.asarray(dcor, dtype=np.float32)


def run(x, y, mm_mode=None, trace=False, tmpdir=None):
    mode = mm_mode if mm_mode in MODES else DEFAULT_MODE
    nc = _get_program(mode)
    in_maps, host = make_in_maps(x, y, mode)
    res = run_bass_kernel_spmd(nc, in_maps, core_ids=list(range(NCORES)),
                               trace=trace, tmpdir=tmpdir)
    outs = [r["out"] for r in res.results]
    colss = [r["cols"] for r in res.results] if mode == "symdr" else None
    st2s = ([(r["st0o"], r["st1o"], r["st2"]) for r in res.results]
            if mode == "symdr" else None)
    return finalize(outs, host, mode, colss, st2s), res


def kernel(x, y):
    val, _ = run(x, y)
    return val


# revision 48
# speedup vs baseline: 1.0224x; 1.0112x over previous
"""Distance-correlation (DcorLoss) kernel for 8 trn2 NeuronCores.

Math: for x, y [n=8192, d=128]:
  a = pairwise_dist(x), b = pairwise_dist(y)   (n x n, symmetric, zero diag)
  A = double_center(a), B = double_center(b)
  dcor = -sqrt(sum(A*B)) / sqrt(sqrt(sum(A*A)) * sqrt(sum(B*B)))

Never materialize A/B:
  sum(A o B) = sum(a o b) - 2/n dot(rs_a, rs_b) + sum(a) sum(b) / n^2
and the squared-distance Frobenius norms have a closed form (host, exact):
  sum_ij dist^2 = 2n sum_i |x_i|^2 - 2 |sum_i x_i|^2
so the device only streams: row sums of a and b (ACT accum), column sums
(PE matmul with ones weights), and sum (a - mu) * b (DVE accum). All
combining is host fp64.

Default mode "symdr" stacks three structural tricks:

1. fp8 DoubleRow matmul (perf_mode=DoubleRow, K=256 virtual): plane 0
   carries the 128 data rows (-2 x_blk^T x gram), plane 1 rows 0..2 carry
   the column-norm hi/lo/lo2 splits against all-ones weight rows. One MM
   per 512-col psum half computes n_i-free sq distances entirely, halving
   PE streaming vs bf16 mains + K=2 norm matmuls.
2. Symmetry: core c computes only local windows 0..4 (its diagonal block
   + 4 cyclic neighbors) = 5/8 of the row-block work. Full-matrix sums
   use sum_full = 2*sum_computed - S(w=0) - S(w=4); full row sums add
   mirrored per-column sums of windows 1..3 (PE ones-matmuls into one
   psum bank at partitions 0/32/64/96), gathered on host.
3. Per-core COLUMN ROTATION: core c's column j is global (j + c*1024)
   mod n, so the diagonal lands in window 0 on every core and the SPMD
   program is identical; the mu^2 diagonal forcing (sqrt NaN-safety)
   costs 2 bf16 matmuls on window-0 tiles only.

Precision: the final sums cancel ~1e8 -> ~1e6, which amplifies any BIAS
~1e4x. bf16-rounded sqrt outputs carry E[delta] ~ -2e-4 -> 5% error, so:
products and row-sum accums run on fp32 ACT outputs (unbiased); only the
column-sum matmuls read separate bf16 copies; global Ra/Rb come from the
unbiased fp32 accum totals; the dot products are mean-centered, which
cancels the constant per-row bias of the mirrored column sums.

ACT (ScalarE) is the bottleneck: 80 sqrt passes at 1 elem/cycle/lane
is ~95 us; PE (DR mains + colsums) and DVE (products + bf16 casts)
overlap underneath.

All operand prep is host-side: inputs arrive as fp8/f32 in final layout;
no on-device casts / norm computation / big reductions.
"""

import numpy as np
import ml_dtypes

import concourse.bass as bass
import concourse.tile as tile
from concourse import bacc, mybir
from concourse.bass_utils import run_bass_kernel_spmd

P = 128            # partitions / d
N = 8192           # points
NCORES = 8
BLK = N // NCORES  # 1024 rows per core
CI_N = BLK // P    # 8 row chunks per core
W = 1024           # column window
JT_N = N // W      # 8 column windows
MU = 16.0          # ~E[pairwise dist] for randn d=128; any constant is exact
RES_W = 24

BF16 = ml_dtypes.bfloat16
F8 = ml_dtypes.float8_e4m3

DEFAULT_MODE = "symdr"
MODES = ("base", "dr", "symdr")
NW = 5             # symdr: local windows 0..4 (diag + 4 cyclic)
CS_ROWS = (0, 32, 64, 96)  # psum partitions for a_h0, a_h1, b_h0, b_h1

_programs = {}


NW_SYM = 5         # sym mode: windows 0..4 (diag + 4 cyclic) per core
NCOL = N  # dram moving-tensor width (full; sym mode reads first 5 windows)


def _build(mode: str):
    dt = mybir.dt
    f32 = dt.float32
    bf = dt.bfloat16
    A = mybir.AluOpType
    AF = mybir.ActivationFunctionType

    f8 = dt.float8e4

    nc = bacc.Bacc("TRN2", target_bir_lowering=False, debug=False,
                   num_devices=NCORES)

    if mode in ("dr", "symdr"):
        # fp8 DoubleRow: K=256 virtual; plane 0 = data rows, plane 1 rows
        # 0..2 = column-norm hi/lo/lo2 (weights = ones there), rest zero.
        # dram layout is window-major so per-window DMAs are contiguous
        # 2 KiB/partition lines instead of strided 1 KiB ones.
        dxm = nc.dram_tensor("xm", [P, JT_N, 2, W], f8,
                             kind="ExternalInput").ap()
        dym = nc.dram_tensor("ym", [P, JT_N, 2, W], f8,
                             kind="ExternalInput").ap()
        dwx = nc.dram_tensor("wx", [P, 2, BLK], f8, kind="ExternalInput").ap()
        dwy = nc.dram_tensor("wy", [P, 2, BLK], f8, kind="ExternalInput").ap()
    else:
        dxT = nc.dram_tensor("xT", [P, N], bf, kind="ExternalInput").ap()
        dyT = nc.dram_tensor("yT", [P, N], bf, kind="ExternalInput").ap()
        dxb = nc.dram_tensor("xblk2", [P, BLK], bf, kind="ExternalInput").ap()
        dyb = nc.dram_tensor("yblk2", [P, BLK], bf, kind="ExternalInput").ap()
        dnfx = nc.dram_tensor("nfx", [2, N], bf, kind="ExternalInput").ap()
        dnfy = nc.dram_tensor("nfy", [2, N], bf, kind="ExternalInput").ap()
    dnbx = nc.dram_tensor("nbx", [P, CI_N], f32, kind="ExternalInput").ap()
    dnby = nc.dram_tensor("nby", [P, CI_N], f32, kind="ExternalInput").ap()
    deye = nc.dram_tensor("eye128", [P, P], bf, kind="ExternalInput").ap()
    dew = nc.dram_tensor("eyewide", [P, 4 * 512], bf, kind="ExternalInput").ap()
    dout = nc.dram_tensor("out", [P, RES_W], f32, kind="ExternalOutput").ap()
    if mode == "symdr":
        dcols = nc.dram_tensor("cols", [P, 3 * 512], f32,
                               kind="ExternalOutput").ap()
        dst01 = [nc.dram_tensor(f"st{q}o", [P, CI_N * NW], f32,
                                kind="ExternalOutput").ap() for q in range(2)]
        dst2 = nc.dram_tensor("st2", [P, CI_N * NW], f32,
                              kind="ExternalOutput").ap()
    n_w = NW if mode == "symdr" else JT_N

    with tile.TileContext(nc) as tc:
        with tc.tile_pool(name="const", bufs=1) as cp, \
             tc.tile_pool(name="psum", bufs=3, space="PSUM") as pp, \
             tc.tile_pool(name="cspsum", bufs=2, space="PSUM") as csp, \
             tc.tile_pool(name="ab", bufs=3) as abp, \
             tc.tile_pool(name="trd", bufs=2) as trd:

            # ── persistent operands, DMA'd in final dtype/layout ──────
            if mode in ("dr", "symdr"):
                xm = cp.tile([P, 2, N], f8, tag="xm")
                ym = cp.tile([P, 2, N], f8, tag="ym")
                wx = cp.tile([P, 2, BLK], f8, tag="wx")
                wy = cp.tile([P, 2, BLK], f8, tag="wy")
            else:
                xTc = cp.tile([P, N], bf, tag="xTc")
                yTc = cp.tile([P, N], bf, tag="yTc")
                xblk2 = cp.tile([P, BLK], bf, tag="xblk2")
                yblk2 = cp.tile([P, BLK], bf, tag="yblk2")
                nfx = cp.tile([2, N], bf, tag="nfx")
                nfy = cp.tile([2, N], bf, tag="nfy")
            nbx = cp.tile([P, CI_N], f32, tag="nbx")
            nby = cp.tile([P, CI_N], f32, tag="nby")
            eye128 = cp.tile([P, P], bf, tag="eye128")
            eyew = cp.tile([P, 4 * 512], bf, tag="eyew")
            ones2 = cp.tile([2, P], bf, tag="ones2")
            nc.vector.memset(ones2[:], 1.0)

            res = cp.tile([P, RES_W], f32, tag="res")
            nc.vector.memset(res[:], 0.0)

            st = [cp.tile([P, CI_N * n_w], f32, tag=f"st{q}", name=f"st{q}")
                  for q in range(3)]
            if mode == "symdr":
                onesP = cp.tile([P, 1], bf, tag="onesP")
                nc.vector.memset(onesP[:], 1.0)
                colsave = cp.tile([P, 3 * 512], f32, tag="colsave")

            # PE warm-up on constant data: release the HAM clock-gate
            # before real matmuls start (cold runs stream at 1.2 GHz).
            # symdr skips it: ACT (not PE) is the bottleneck, so warm-up
            # only delays the first main matmuls behind the DMA wave.
            wur = cp.tile([2, 512], bf, tag="wur")
            nc.vector.memset(wur[:], 0.0)
            n_warm = 0 if mode == "symdr" else 24
            for _ in range(n_warm):
                wt = pp.tile([P, W], f32, tag="ps")
                nc.tensor.matmul(wt[:, 0:512], ones2[:], wur[:],
                                 start=True, stop=True)
            # trigger the sqrt ACT_TABLE_LOADs (~2.7us) during the DMA wait
            # instead of on the first real tile; same bias/accum signature
            # as the real tiles so every needed table set loads now
            tldu = cp.tile([1, 8], f32, tag="tldu")
            tlda = cp.tile([1, 1], f32, tag="tlda")
            tldb = cp.tile([1, 1], f32, tag="tldb")
            nc.vector.memset(tldb[:], 0.0)
            nc.scalar.activation(tldu[:], wur[0:1, 0:8], AF.Sqrt,
                                 bias=tldb[:], accum_out=tlda[:])

            # small operands first, then per-window slices of the big
            # moving tensors so window-0 compute starts ASAP
            if mode in ("dr", "symdr"):
                # the scalar queue carries the ACTIVATE stream: only the
                # few transfers that gate tile 0 may dispatch there; all
                # bulk DMAs go on sync so their ~640ns dispatches never
                # delay the first sqrt
                w_first = 1 if mode == "symdr" else 0
                sl0 = bass.ts(w_first, W)
                nc.sync.dma_start(wx[:], dwx[:])
                nc.scalar.dma_start(wy[:], dwy[:])
                # first-window moving tensors split by partition halves
                # across both queues: their transfer time gates tile 0
                for lo, hi in ((0, 64), (64, 128)):
                    eng = nc.sync if lo == 0 else nc.scalar
                    eng.dma_start(xm[lo:hi, :, sl0], dxm[lo:hi, w_first])
                for lo, hi in ((0, 64), (64, 128)):
                    eng = nc.scalar if lo == 0 else nc.sync
                    eng.dma_start(ym[lo:hi, :, sl0], dym[lo:hi, w_first])
            else:
                nc.sync.dma_start(xblk2[:], dxb[:])
                nc.sync.dma_start(yblk2[:], dyb[:])
            nc.sync.dma_start(nbx[:], dnbx[:])
            nc.scalar.dma_start(nby[:], dnby[:])
            nc.sync.dma_start(eye128[:], deye[:])
            if mode in ("dr", "symdr"):
                order = [2, 3, 4, 0] if mode == "symdr" else range(1, n_w)
                for w in order:
                    sl = bass.ts(w, W)
                    nc.sync.dma_start(xm[:, :, sl], dxm[:, w])
                    nc.sync.dma_start(ym[:, :, sl], dym[:, w])
                nc.sync.dma_start(eyew[:], dew[:])
            else:
                nc.sync.dma_start(eyew[:], dew[:])
                nc.sync.dma_start(nfx[:], dnfx[:])
                nc.sync.dma_start(nfy[:], dnfy[:])
                for w in range(JT_N):
                    sl = bass.ts(w, W)
                    nc.sync.dma_start(xTc[:, sl], dxT[:, sl])
                    nc.sync.dma_start(yTc[:, sl], dyT[:, sl])

            # ── main loop ─────────────────────────────────────────────
            # symdr: window 1 first (no diag fix -> shortest dependency
            # chain to the first ACTIVATE); diag window 0 last, where the
            # PE has slack under the ACT-bound steady state
            w_order = [1, 2, 3, 4, 0] if mode == "symdr" else range(n_w)
            for w in w_order:
                do_cs = mode == "symdr" and 1 <= w <= 3
                if do_cs:
                    cs = csp.tile([P, 512], f32, tag="cs")
                for ci in range(CI_N):
                    col = ci * n_w + w
                    h_diag = ci // 4
                    psA = pp.tile([P, W], f32, tag="ps")
                    psB = pp.tile([P, W], f32, tag="ps")
                    if mode in ("dr", "symdr"):
                        # one fp8 DoubleRow matmul per half: data + norm
                        # rows in a single K=256 pass; diag fix right after
                        # its own psum's matmuls so the ACT dependency
                        # chain stays per-psum
                        for ps_, ww, mov in ((psA, wx, xm), (psB, wy, ym)):
                            for h in range(2):
                                last = not (w == 0 and h == h_diag)
                                nc.tensor.matmul(
                                    ps_[:, bass.ds(h * 512, 512)],
                                    ww[:, :, bass.ts(ci, P)],
                                    mov[:, :, bass.ds(w * W + h * 512, 512)],
                                    start=True, stop=last,
                                    perf_mode=mybir.MatmulPerfMode.DoubleRow)
                            if w == 0 and mode == "symdr":
                                nc.tensor.matmul(
                                    ps_[:, bass.ds(h_diag * 512, 512)],
                                    eye128[:],
                                    eyew[:, bass.ts(ci % 4, 512)],
                                    start=False, stop=True)
                    else:
                        # mains (grouped by weights), then norm rows (shared
                        # ones2 weights), then diag fix (window 0 only)
                        for ps_, blk2, full in ((psA, xblk2, xTc),
                                                (psB, yblk2, yTc)):
                            for h in range(2):
                                nc.tensor.matmul(
                                    ps_[:, bass.ds(h * 512, 512)],
                                    blk2[:, bass.ts(ci, P)],
                                    full[:, bass.ds(w * W + h * 512, 512)],
                                    start=True, stop=False)
                        for ps_, nf in ((psA, nfx), (psB, nfy)):
                            for h in range(2):
                                last = not (w == 0 and h == h_diag)
                                nc.tensor.matmul(
                                    ps_[:, bass.ds(h * 512, 512)], ones2[:],
                                    nf[:, bass.ds(w * W + h * 512, 512)],
                                    start=False, stop=last)
                    if w == 0 and mode != "symdr":
                        for ps_ in (psA, psB):
                            nc.tensor.matmul(
                                ps_[:, bass.ds(h_diag * 512, 512)],
                                eye128[:],
                                eyew[:, bass.ts(ci % 4, 512)],
                                start=False, stop=True)

                    aT = abp.tile([P, W], f32, tag="a")
                    bT = abp.tile([P, W], f32, tag="b")
                    nc.scalar.activation(aT[:], psA[:], AF.Sqrt,
                                         bias=nbx[:, ci:ci + 1],
                                         accum_out=st[0][:, col:col + 1])
                    nc.scalar.activation(bT[:], psB[:], AF.Sqrt,
                                         bias=nby[:, ci:ci + 1],
                                         accum_out=st[1][:, col:col + 1])
                    if do_cs:
                        # bf16 copies feed the column-sum matmuls; the fp32
                        # originals feed the product pass (bf16 rounding
                        # bias is amplified ~1e4x by cancellation in the
                        # final sums, so rs/pab paths must stay fp32)
                        a16 = trd.tile([P, W], bf, tag="a16")
                        b16 = trd.tile([P, W], bf, tag="b16")
                        nc.vector.tensor_copy(a16[:], aT[:])
                        nc.vector.tensor_copy(b16[:], bT[:])
                        # column sums for mirrored row sums: one psum bank,
                        # streams at psum partitions 0/32/64/96
                        for r, t_ in ((0, a16), (64, b16)):
                            for h in range(2):
                                nc.tensor.matmul(
                                    cs[r + 32 * h:r + 32 * h + 1, 0:512],
                                    onesP[:],
                                    t_[:, bass.ds(h * 512, 512)],
                                    start=(ci == 0), stop=(ci == CI_N - 1),
                                    tile_position=(0, r + 32 * h))
                    t0 = trd.tile([P, W], bf, tag="t")
                    nc.vector.scalar_tensor_tensor(
                        t0[:], aT[:], MU, bT[:], op0=A.subtract, op1=A.mult,
                        accum_out=st[2][:, col:col + 1])
                if do_cs:
                    csl = bass.ts(w - 1, 512)
                    nc.vector.tensor_copy(colsave[:, csl], cs[:])
                    nc.sync.dma_start(dcols[:, csl], colsave[:, csl])

            # ── epilogue: fold windows, ship result ───────────────────
            if mode == "symdr":
                nc.sync.dma_start(dst01[0][:], st[0][:])
                nc.scalar.dma_start(dst01[1][:], st[1][:])
                nc.sync.dma_start(dst2[:], st[2][:])
                nc.scalar.dma_start(dout[:], res[:])
            else:
                for q in range(3):
                    for ci in range(CI_N):
                        o = q * CI_N + ci
                        nc.vector.tensor_reduce(res[:, o:o + 1],
                                                st[q][:, bass.ts(ci, n_w)],
                                                axis=mybir.AxisListType.X,
                                                op=A.add)
                nc.sync.dma_start(dout[:], res[:])

    nc.compile()
    return nc


def _get_program(mode: str):
    if mode not in _programs:
        _programs[mode] = _build(mode)
    return _programs[mode]


def make_in_maps(x: np.ndarray, y: np.ndarray, mode: str = "bf16"):
    x = np.asarray(x, np.float32)
    y = np.asarray(y, np.float32)
    QD = F8 if mode in ("dr", "symdr") else BF16
    xb = x.astype(QD)
    yb = y.astype(QD)

    eye = (np.eye(P) * MU).astype(BF16)
    ew = np.zeros((P, 4 * 512), BF16)
    for k in range(4):
        for p in range(P):
            ew[p, k * 512 + k * P + p] = BF16(MU)

    def norms_split(vb, parts):
        n64 = (vb.astype(np.float64) ** 2).sum(axis=1)
        rows, rem = [], n64.copy()
        for _ in range(parts):
            r = rem.astype(QD)
            rows.append(r)
            rem = rem - r.astype(np.float64)
        return n64, np.stack(rows).astype(QD)

    nparts = 3 if mode in ("dr", "symdr") else 2
    nx64, nfx = norms_split(xb, nparts)
    ny64, nfy = norms_split(yb, nparts)

    xT = np.ascontiguousarray(xb.T)           # [128, 8192]
    yT = np.ascontiguousarray(yb.T)
    xT2 = np.concatenate([xT, xT], axis=1)    # for cheap rotation slicing
    yT2 = np.concatenate([yT, yT], axis=1)
    nfx2 = np.concatenate([nfx, nfx], axis=1)
    nfy2 = np.concatenate([nfy, nfy], axis=1)

    in_maps = []
    for c in range(NCORES):
        o = c * BLK
        xTr = np.ascontiguousarray(xT2[:, o:o + N])
        yTr = np.ascontiguousarray(yT2[:, o:o + N])
        im = {
            "nbx": np.ascontiguousarray(
                nx64[o:o + BLK].reshape(CI_N, P).T.astype(np.float32)),
            "nby": np.ascontiguousarray(
                ny64[o:o + BLK].reshape(CI_N, P).T.astype(np.float32)),
            "eye128": eye,
            "eyewide": ew,
        }
        if mode in ("dr", "symdr"):
            def moving(vT, nf2):
                m = np.zeros((P, 2, N), QD)
                m[:, 0, :] = vT
                m[0:3, 1, :] = nf2[:, o:o + N]
                # window-major dram layout: [P, JT_N, 2, W]
                return np.ascontiguousarray(
                    m.reshape(P, 2, JT_N, W).transpose(0, 2, 1, 3))

            def weights(vT):
                ww = np.zeros((P, 2, BLK), QD)
                ww[:, 0, :] = QD(-2.0) * vT[:, 0:BLK]
                ww[0:3, 1, :] = QD(1.0)
                return ww

            im.update({"xm": moving(xTr, nfx2), "ym": moving(yTr, nfy2),
                       "wx": weights(xTr), "wy": weights(yTr)})
        else:
            im.update({
                "xT": xTr, "yT": yTr,
                "xblk2": np.ascontiguousarray(BF16(-2.0) * xTr[:, 0:BLK]),
                "yblk2": np.ascontiguousarray(BF16(-2.0) * yTr[:, 0:BLK]),
                "nfx": np.ascontiguousarray(nfx2[:, o:o + N]),
                "nfy": np.ascontiguousarray(nfy2[:, o:o + N]),
            })
        in_maps.append(im)
    host = {"xb64": xb.astype(np.float64), "yb64": yb.astype(np.float64)}
    return in_maps, host


def finalize(outs, host, mode="base", colss=None, st2s=None):
    """outs: 8 x [128, 24] f32 -> scalar dcor (host fp64).

    Cols per core: rs_a 0:8 | rs_b 8:16 | pab 16:24, laid out [p, ci] for
    global row c*1024 + ci*128 + p. Device row sums include the forced
    diagonal entry sqrt(mu^2) = mu (true diag of a distance matrix is 0).

    symdr: each core computed local windows 0..4 only. Full sums over the
    symmetric matrices: sum_full = 2*sum_computed - S(w=0) - S(w=4); full
    row sums add mirrored column sums from windows 1..3 of cores bi-1..bi-3.
    """
    n = float(N)
    rs_a = np.empty(N, np.float64)
    rs_b = np.empty(N, np.float64)
    pab = 0.0

    # exact Frobenius norms of the quantized-point distance matrices
    def sq_frob(v64):
        s = v64.sum(axis=0)
        return 2.0 * n * (v64 * v64).sum() - 2.0 * np.dot(s, s)

    sq_a = sq_frob(host["xb64"])
    sq_b = sq_frob(host["yb64"])

    if mode == "symdr":
        # per-row strip sums + per-window totals (for 2*S - S0 - S4)
        tot = np.zeros((2, 3), np.float64)  # [a/b][all, w0, w4]
        for c in range(NCORES):
            for q, rs in ((0, rs_a), (1, rs_b)):
                sq_ = np.asarray(st2s[c][q], np.float64).reshape(P, CI_N, NW)
                rs[c * BLK:(c + 1) * BLK] = (
                    sq_.sum(axis=2).T.ravel())
                tot[q] += (sq_.sum(), sq_[:, :, 0].sum(), sq_[:, :, 4].sum())
        # mirrored row-sum contributions from columns of windows 1..3
        VA = np.empty((NCORES, 3, BLK), np.float64)
        VB = np.empty((NCORES, 3, BLK), np.float64)
        for c, cl in enumerate(colss):
            cl = np.asarray(cl, np.float64)
            for wp in range(1, 4):
                sl = slice((wp - 1) * 512, wp * 512)
                VA[c, wp - 1] = np.concatenate([cl[0, sl], cl[32, sl]])
                VB[c, wp - 1] = np.concatenate([cl[64, sl], cl[96, sl]])
        for bi in range(NCORES):
            for wp in range(1, 4):
                rs_a[bi * BLK:(bi + 1) * BLK] += VA[(bi - wp) % NCORES, wp - 1]
                rs_b[bi * BLK:(bi + 1) * BLK] += VB[(bi - wp) % NCORES, wp - 1]
        # pab over the full matrix: 2*computed - S(w0) - S(w4)
        p_all = p_w0 = p_w4 = 0.0
        for s2 in [st2s[c][2] for c in range(NCORES)]:
            s2 = np.asarray(s2, np.float64).reshape(P, CI_N, NW)
            p_all += s2.sum()
            p_w0 += s2[:, :, 0].sum()
            p_w4 += s2[:, :, 4].sum()
        pab = 2.0 * p_all - p_w0 - p_w4
        # unbiased global sums (fp32 accum path; removes forced diag)
        Ra = 2.0 * tot[0, 0] - tot[0, 1] - tot[0, 2] - n * MU
        Rb = 2.0 * tot[1, 0] - tot[1, 1] - tot[1, 2] - n * MU
        sa = rs_a - MU         # per-row (mirror part carries tiny bf16 bias;
        sb = rs_b - MU         # centered dots below are immune to it)
        sat = sa - Ra / n
        sbt = sb - Rb / n
        sum_ab = pab + MU * Rb
        sumAB = sum_ab - 2.0 * np.dot(sat, sbt) / n - Ra * Rb / n**2
        sumAA = sq_a - 2.0 * np.dot(sat, sat) / n - Ra * Ra / n**2
        sumBB = sq_b - 2.0 * np.dot(sbt, sbt) / n - Rb * Rb / n**2
        inv_n2 = 1.0 / (n * n)
        return np.asarray(
            -np.sqrt(sumAB * inv_n2)
            / np.sqrt(np.sqrt(sumAA * inv_n2) * np.sqrt(sumBB * inv_n2)),
            dtype=np.float32)

    for c, o in enumerate(outs):
        o = np.asarray(o, np.float64)
        rs_a[c * BLK:(c + 1) * BLK] = o[:, 0:CI_N].T.ravel()
        rs_b[c * BLK:(c + 1) * BLK] = o[:, CI_N:2 * CI_N].T.ravel()
        pab += o[:, 2 * CI_N:3 * CI_N].sum()

    sa = rs_a - MU             # true (zero-diag) row sums of a
    sb = rs_b - MU
    Ra = sa.sum()
    Rb = sb.sum()

    # device pab = sum (a - mu) * b; forced diag contributes (mu-mu)*mu = 0,
    # matching the true diag contribution (0 - mu) * 0 = 0 exactly.
    sum_ab = pab + MU * Rb

    sumAB = sum_ab - 2.0 * np.dot(sa, sb) / n + Ra * Rb / n**2
    sumAA = sq_a - 2.0 * np.dot(sa, sa) / n + Ra * Ra / n**2
    sumBB = sq_b - 2.0 * np.dot(sb, sb) / n + Rb * Rb / n**2

    inv_n2 = 1.0 / (n * n)
    dcor = (-np.sqrt(sumAB * inv_n2)
            / np.sqrt(np.sqrt(sumAA * inv_n2) * np.sqrt(sumBB * inv_n2)))
    return np.asarray(dcor, dtype=np.float32)


def run(x, y, mm_mode=None, trace=False, tmpdir=None):
    mode = mm_mode if mm_mode in MODES else DEFAULT_MODE
    nc = _get_program(mode)
    in_maps, host = make_in_maps(x, y, mode)
    res = run_bass_kernel_spmd(nc, in_maps, core_ids=list(range(NCORES)),
                               trace=trace, tmpdir=tmpdir)
    outs = [r["out"] for r in res.results]
    colss = [r["cols"] for r in res.results] if mode == "symdr" else None
    st2s = ([(r["st0o"], r["st1o"], r["st2"]) for r in res.results]
            if mode == "symdr" else None)
    return finalize(outs, host, mode, colss, st2s), res


def kernel(x, y):
    val, _ = run(x, y)
    return val
